# revision 3
# baseline (speedup 1.0000x reference)
"""Trainium2 Bass kernel for nn_BatchMatMulModule.

Computes out = einsum("bnij,bmj->bnmi", x, y) with
  x: [4, 64, 3, 3] f32, y: [4, 100000, 3] f32 -> out: [4, 64, 100000, 3] f32.

The output (307 MB) dwarfs the inputs (4.8 MB); per-core store floor is
~107 us (38.4 MB at ~358 GB/s HBM per NeuronCore). The v1 kernel was
DVE-bound at ~165 us because both accumulate passes were fp32
scalar_tensor_tensor ops (1x mode, ~1.04 ns/elem each). This version
restructures the compute around DVE perf modes:

- y is shipped from the host as bf16 *planes* (yt = y[b].T, [3, 100000])
  so every compute read is contiguous; x scalars stay fp32 (scalar
  operands are exempt from the 2x-mode dtype rule).
- Per output element: two bf16 products (DVE tensor_scalar runs 4x: 0.26
  ns/elem; or ACT activation: 1x @1.2 GHz, 0.83 ns/elem), one bf16
  tensor_tensor add (DVE 2x: 0.52 ns/elem), and one final
  scalar_tensor_tensor that fuses the third product with the accumulate
  and the strided fp32 interleave write (1x, 1.04 ns/elem - the
  interleave into the [.., m, 3] output layout forces 1x regardless, so
  only ONE such pass remains vs v1's two).
- Chains are assigned per-engine via CHAIN_CODES to balance ACT vs DVE
  (vs optionally GPSIMD) busy time.

Sharding: core c handles b = c // 2, n in [32 * (c % 2), ...) as in v1.
Per core, partition p = (a in 0..8, s in 0..16); group g covers n =
g*8 + a; segment s covers y rows [s*6250, (s+1)*6250); each unit
(g, h in 0..2) computes rows bounds[h]..bounds[h+1] (3124/3126 split so
bf16 packed modes keep even element counts and 4B alignment).
"""

import numpy as np
import ml_dtypes

import concourse.bacc as bacc
import concourse.mybir as mybir
from concourse.bass_utils import run_bass_kernel_spmd
from concourse.tile import TileContext

N_CORES = 8
P = 128
N_PER_CORE = 32
N_SUB = 8            # n values per group (partition-major)
SEGS = 16            # m segments per partition group
N_GROUPS = N_PER_CORE // N_SUB   # 4
M = 100000
ROWS = M // SEGS     # 6250 rows per segment (even)
BOUNDS = (0, 3124, ROWS)  # unit halves; both chunks even-sized
N_HALVES = len(BOUNDS) - 1

TRACE = False
LAST = None

_CACHED_NC = None

# Per-chain engine assignment. Chain index = (g * N_HALVES + h) * 3 + i.
# Code = 5 chars (p0, p1, add, fin, p2):
#   p0/p1: engine for the j=0 / j=1 products: 'A' (ACT) or 'V' (DVE ts)
#   add:   engine for the j0+j1 add: 'V' (DVE tt) or 'P' (GPSIMD tt)
#   fin:   final op: 'V' = DVE stt (fuses j2 product, writes fp32 strided)
#          'P' = GPSIMD tt (needs explicit j2 product, see p2)
#   p2:    engine for the j=2 product when fin == 'P' ('A'/'V'), else '-'


def _spread(counts):
    """Build a 24-chain code list interleaving the given {code: count}."""
    codes = []
    for code, cnt in counts.items():
        codes.extend([code] * cnt)
    assert len(codes) == N_GROUPS * N_HALVES * 3, len(codes)
    # interleave so consecutive chains mix engine types
    out = []
    step = 7  # coprime with 24
    idx = 0
    taken = [False] * len(codes)
    for _ in range(len(codes)):
        while taken[idx]:
            idx = (idx + 1) % len(codes)
        out.append(codes[idx])
        taken[idx] = True
        idx = (idx + step) % len(codes)
    return out

# Config A (no GPSIMD): balance ACT (2 products on 22 chains) vs DVE.
CONFIG_A = _spread({"AAVV-": 22, "VVVV-": 2})
# Config B (with GPSIMD adds/finals).
CONFIG_B = _spread({"AVVV-": 8, "AAVV-": 6, "AAPV-": 6, "AVVPV": 4})

CHAIN_CODES = CONFIG_A


def build_bass(reps: int = 1, ops_mode: str = "full", chain_codes=None):
    if chain_codes is None:
        chain_codes = CHAIN_CODES
    nc = bacc.Bacc(
        "TRN2",
        debug=False,
        enable_asserts=False,
        target_bir_lowering=False,
        num_devices=N_CORES,
    )
    f32 = mybir.dt.float32
    bf16 = mybir.dt.bfloat16
    mult = mybir.AluOpType.mult
    add = mybir.AluOpType.add
    copy = mybir.ActivationFunctionType.Copy

    # xs[p = a*SEGS + s, col = g*9 + i*3 + j] = x[b, g*8 + a, i, j]
    xs = nc.dram_tensor("xs", [P, N_GROUPS * 9], f32, kind="ExternalInput").ap()
    # yt[j, m] = y[b, m, j]  (bf16 planes)
    yt = nc.dram_tensor("yt", [3, M], bf16, kind="ExternalInput").ap()
    out = nc.dram_tensor("out", [N_PER_CORE, M, 3], f32, kind="ExternalOutput").ap()

    with TileContext(nc) as tc:
        with (
            tc.tile_pool(name="const", bufs=1) as cpool,
            tc.tile_pool(name="tmpp", bufs=2) as tpool,
            tc.tile_pool(name="outp", bufs=2) as opool,
        ):
            xsb = cpool.tile([P, N_GROUPS * 9], f32)
            nc.sync.dma_start(out=xsb[:], in_=xs)

            # y resident in SBUF as bf16 planes: partition (a, s) holds
            # [j, t] = yt[j, s*ROWS + t]; replicated over the 8 a-groups.
            # Loaded in halves so unit (g=0, h=0) can start early.
            y_tile = cpool.tile([P, 3 * ROWS], bf16)
            yv = y_tile.rearrange("p (j t) -> p j t", j=3)
            y_src = yt.rearrange("j (s t) -> j s t", s=SEGS)  # [3, 16, 6250]
            for h in range(N_HALVES):
                t0, t1 = BOUNDS[h], BOUNDS[h + 1]
                for j in range(3):
                    nc.sync.dma_start(
                        out=yv[:, j, t0:t1],
                        in_=y_src[j, :, t0:t1].unsqueeze(0)
                        .to_broadcast((N_SUB, SEGS, t1 - t0)),
                    )

            units = [(g, h) for g in range(N_GROUPS) for h in range(N_HALVES)]
            units = units * reps

            def emit_unit(u, g, h):
                t0, t1 = BOUNDS[h], BOUNDS[h + 1]
                nt = t1 - t0
                ot = opool.tile([P, nt * 3], f32, name="ot", tag="ot")
                ov = ot.rearrange("p (t i) -> p t i", i=3)
                ys = [yv[:, j, t0:t1] for j in range(3)]
                prods = []
                for i in range(3):
                    code = chain_codes[((g * N_HALVES + h) * 3 + i) % 24]
                    c = g * 9 + i * 3
                    a = tpool.tile([P, nt], bf16, name=f"a{i}", tag=f"a{i}")
                    b = tpool.tile([P, nt], bf16, name=f"b{i}", tag=f"b{i}")
                    for slot, (eng, j, dst) in enumerate(
                        [(code[0], 0, a), (code[1], 1, b)]
                    ):
                        if eng == "A":
                            nc.scalar.activation(
                                out=dst[:], in_=ys[j], func=copy,
                                scale=xsb[:, c + j:c + j + 1])
                        else:
                            nc.vector.tensor_scalar(
                                out=dst[:], in0=ys[j],
                                scalar1=xsb[:, c + j:c + j + 1], scalar2=None,
                                op0=mult)
                    prods.append((code, c, a, b))
                for i in range(3):
                    code, c, a, b = prods[i]
                    if code[2] == "V":
                        nc.vector.tensor_tensor(out=a[:], in0=a[:], in1=b[:],
                                                op=add)
                    else:
                        nc.gpsimd.tensor_tensor(out=a[:], in0=a[:], in1=b[:],
                                                op=add)
                for i in range(3):
                    code, c, a, b = prods[i]
                    if code[3] == "V":
                        # out_i = (y2 * x2) + (j0 + j1): fuses the third
                        # product into the strided fp32 interleave write.
                        nc.vector.scalar_tensor_tensor(
                            out=ov[:, :, i], in0=ys[2],
                            scalar=xsb[:, c + 2:c + 3], in1=a[:],
                            op0=mult, op1=add)
                    else:
                        b2 = tpool.tile([P, nt], bf16, name=f"c{i}", tag=f"c{i}")
                        if code[4] == "A":
                            nc.scalar.activation(
                                out=b2[:], in_=ys[2], func=copy,
                                scale=xsb[:, c + 2:c + 3])
                        else:
                            nc.vector.tensor_scalar(
                                out=b2[:], in0=ys[2],
                                scalar1=xsb[:, c + 2:c + 3], scalar2=None,
                                op0=mult)
                        nc.gpsimd.tensor_tensor(
                            out=ov[:, :, i], in0=a[:], in1=b2[:], op=add)
                dst = out[g * N_SUB:(g + 1) * N_SUB, :, :].rearrange(
                    "a (s t) i -> (a s) t i", s=SEGS
                )[:, t0:t1, :]
                nc.sync.dma_start(out=dst, in_=ot[:])

            def emit_unit_none(u, g, h):
                t0, t1 = BOUNDS[h], BOUNDS[h + 1]
                nt = t1 - t0
                ot = opool.tile([P, nt * 3], f32, name="ot", tag="ot")
                nc.vector.memset(ot[:], 0.0)
                dst = out[g * N_SUB:(g + 1) * N_SUB, :, :].rearrange(
                    "a (s t) i -> (a s) t i", s=SEGS
                )[:, t0:t1, :]
                nc.sync.dma_start(out=dst, in_=ot[:])

            for u, (g, h) in enumerate(units):
                if ops_mode == "full":
                    emit_unit(u, g, h)
                elif ops_mode == "none":
                    emit_unit_none(u, g, h)
                else:
                    raise ValueError(ops_mode)
    nc.compile()
    return nc


def _make_in_maps(x, y):
    x_flat = x.reshape(256, 3, 3)
    in_maps = []
    for c in range(N_CORES):
        b = c // 2
        xl = x_flat[c * N_PER_CORE:(c + 1) * N_PER_CORE]  # [32, 3, 3]
        per_a = xl.reshape(N_GROUPS, N_SUB, 9).transpose(1, 0, 2)  # [a, g, 9]
        per_a = per_a.reshape(N_SUB, N_GROUPS * 9)
        xsb_np = np.ascontiguousarray(np.repeat(per_a, SEGS, axis=0))  # [128, 36]
        yt_np = np.ascontiguousarray(y[b].T).astype(ml_dtypes.bfloat16)
        in_maps.append({"xs": xsb_np, "yt": yt_np})
    return in_maps


def kernel(x: np.ndarray, y: np.ndarray) -> np.ndarray:
    global LAST, _CACHED_NC
    x = np.ascontiguousarray(x, dtype=np.float32)
    y = np.ascontiguousarray(y, dtype=np.float32)
    assert x.shape == (4, 64, 3, 3) and y.shape == (4, 100000, 3)

    if _CACHED_NC is None:
        _CACHED_NC = build_bass()
    nc = _CACHED_NC

    in_maps = _make_in_maps(x, y)
    res = run_bass_kernel_spmd(
        nc, in_maps, core_ids=list(range(N_CORES)), trace=TRACE,
    )
    LAST = res
    out = np.concatenate([r["out"] for r in res.results], axis=0)
    return out.reshape(4, 64, 100000, 3)


def _prepare_exec(nc, in_maps):
    """Build a jitted 8-core executor for `nc` with device-resident inputs."""
    import jax
    import concourse.mybir as mybir_
    from jax.experimental.shard_map import shard_map
    from jax.sharding import Mesh, NamedSharding, PartitionSpec
    from concourse.bass2jax import (
        _bass_exec_p, install_neuronx_cc_hook, partition_id_tensor,
    )

    install_neuronx_cc_hook()
    partition_name = nc.partition_id_tensor.name if nc.partition_id_tensor else None
    in_names, out_names, out_avals, zero_outs = [], [], [], []
    for alloc in nc.m.functions[0].allocations:
        if not isinstance(alloc, mybir_.MemoryLocationSet):
            continue
        name = alloc.memorylocations[0].name
        if alloc.kind == "ExternalInput":
            if name != partition_name:
                in_names.append(name)
        elif alloc.kind == "ExternalOutput":
            shape = tuple(alloc.tensor_shape)
            dtype = mybir_.dt.np(alloc.dtype)
            out_names.append(name)
            out_avals.append(jax.core.ShapedArray(shape, dtype))
            zero_outs.append(np.zeros(shape, dtype))
    n_params = len(in_names)
    n_outs = len(out_names)
    all_names = in_names + out_names + ([partition_name] if partition_name else [])

    def _body(*args):
        operands = list(args)
        if partition_name is not None:
            operands.append(partition_id_tensor())
        outs = _bass_exec_p.bind(
            *operands,
            out_avals=tuple(out_avals),
            in_names=tuple(all_names),
            out_names=tuple(out_names),
            lowering_input_output_aliases=(),
            sim_require_finite=True,
            sim_require_nnan=True,
            nc=nc,
        )
        return tuple(outs)

    devices = jax.devices()[:N_CORES]
    mesh = Mesh(np.asarray(devices), ("core",))
    spec = PartitionSpec("core")
    sharded = jax.jit(
        shard_map(
            _body, mesh=mesh, in_specs=(spec,) * (n_params + n_outs),
            out_specs=(spec,) * n_outs, check_rep=False,
        ),
        donate_argnums=tuple(range(n_params, n_params + n_outs)),
        keep_unused=True,
    )
    sh = NamedSharding(mesh, spec)
    ins_dev = [
        jax.device_put(
            np.concatenate([np.asarray(m[name]) for m in in_maps], axis=0), sh
        )
        for name in in_names
    ]
    zeros = [
        jax.device_put(
            np.zeros((N_CORES * z.shape[0], *z.shape[1:]), z.dtype), sh
        )
        for z in zero_outs
    ]

    def run_once(outs):
        res = sharded(*ins_dev, *outs)
        jax.block_until_ready(res)
        return list(res)

    return run_once, zeros


def bench(x, y, reps_pair=(9, 65), samples=24, ops_mode="full", chain_codes=None):
    """Measure steady-state per-workload HW time by differencing kernels
    that run the workload `reps_pair[0]` vs `reps_pair[1]` times."""
    import time
    x = np.ascontiguousarray(x, dtype=np.float32)
    y = np.ascontiguousarray(y, dtype=np.float32)
    in_maps = _make_in_maps(x, y)
    times = {}
    for reps in reps_pair:
        nc = build_bass(reps=reps, ops_mode=ops_mode, chain_codes=chain_codes)
        run_once, zeros = _prepare_exec(nc, in_maps)
        outs = run_once(zeros)  # compile + warm
        ts = []
        for _ in range(samples):
            t0 = time.perf_counter()
            outs = run_once(outs)
            ts.append(time.perf_counter() - t0)
        ts.sort()
        times[reps] = ts[len(ts) // 2]
        print(f"reps={reps}: med {times[reps]*1e3:.2f} ms  min {ts[0]*1e3:.2f}  "
              f"all {[f'{t*1e3:.1f}' for t in ts]}")
    r1, r2 = reps_pair
    per_iter_s = (times[r2] - times[r1]) / (r2 - r1)
    return per_iter_s * 1e9


# revision 8
# speedup vs baseline: 2.6364x; 2.6364x over previous
"""Trainium2 Bass kernel for nn_BatchMatMulModule.

Computes out = einsum("bnij,bmj->bnmi", x, y) with
  x: [4, 64, 3, 3] f32, y: [4, 100000, 3] f32 -> out: [4, 64, 100000, 3] f32.

The output (307 MB) dwarfs the inputs (4.8 MB); per-core store floor is
~107 us (38.4 MB at ~358 GB/s HBM per NeuronCore). The v1 kernel was
DVE-bound at ~165 us because both accumulate passes were fp32
scalar_tensor_tensor ops (1x mode, ~1.04 ns/elem each). This version
restructures the compute around DVE perf modes:

- y is shipped from the host as bf16 *planes* (yt = y[b].T, [3, 100000])
  so every compute read is contiguous; x scalars stay fp32 (scalar
  operands are exempt from the 2x-mode dtype rule).
- Per output element: two bf16 products (DVE tensor_scalar runs 4x: 0.26
  ns/elem; or ACT activation: 1x @1.2 GHz, 0.83 ns/elem), one bf16
  tensor_tensor add (DVE 2x: 0.52 ns/elem), and one final
  scalar_tensor_tensor that fuses the third product with the accumulate
  and the strided fp32 interleave write (1x, 1.04 ns/elem - the
  interleave into the [.., m, 3] output layout forces 1x regardless, so
  only ONE such pass remains vs v1's two).
- Chains are assigned per-engine via CHAIN_CODES to balance ACT vs DVE
  (vs optionally GPSIMD) busy time.

Sharding: core c handles b = c // 2, n in [32 * (c % 2), ...) as in v1.
Per core, partition p = (a in 0..8, s in 0..16); group g covers n =
g*8 + a; segment s covers y rows [s*6250, (s+1)*6250); each unit
(g, h in 0..2) computes rows bounds[h]..bounds[h+1] (3124/3126 split so
bf16 packed modes keep even element counts and 4B alignment).
"""

import numpy as np
import ml_dtypes

import concourse.bacc as bacc
import concourse.mybir as mybir
from concourse.bass_utils import run_bass_kernel_spmd
from concourse.tile import TileContext

N_CORES = 8
P = 128
N_PER_CORE = 32
N_SUB = 8            # n values per group (partition-major)
SEGS = 16            # m segments per partition group
N_GROUPS = N_PER_CORE // N_SUB   # 4
M = 100000
ROWS = M // SEGS     # 6250 rows per segment (even)
BOUNDS = (0, 3124, ROWS)  # unit halves; both chunks even-sized
N_HALVES = len(BOUNDS) - 1

TRACE = False
LAST = None

_CACHED_NC = None

# Per-chain engine assignment. Chain index = (g * N_HALVES + h) * 3 + i.
# Code = 5 chars (p0, p1, add, fin, p2):
#   p0/p1: engine for the j=0 / j=1 products: 'A' (ACT) or 'V' (DVE ts)
#   add:   engine for the j0+j1 add: 'V' (DVE tt) or 'P' (GPSIMD tt)
#   fin:   final op producing ov[:, :, i] (fp32, stride-3 interleave):
#          'V' = DVE stt (fuses j2 product into the 1x strided write)
#          'P' = GPSIMD tt a+b2 (needs explicit j2 product, see p2)
#          'A' = DVE tt a+b2 -> v (bf16 2x), then ACT strided upcast copy
#          'Q' = like 'A' but the strided copy runs on GPSIMD
#   p2:    engine for the j=2 product when fin != 'V' ('A'/'V'), else '-'


def _spread(counts):
    """Build a 24-chain code list interleaving the given {code: count}."""
    codes = []
    for code, cnt in counts.items():
        codes.extend([code] * cnt)
    assert len(codes) == N_GROUPS * N_HALVES * 3, len(codes)
    # interleave so consecutive chains mix engine types
    out = []
    step = 7  # coprime with 24
    idx = 0
    taken = [False] * len(codes)
    for _ in range(len(codes)):
        while taken[idx]:
            idx = (idx + 1) % len(codes)
        out.append(codes[idx])
        taken[idx] = True
        idx = (idx + step) % len(codes)
    return out

# Config A (no GPSIMD): balance ACT (2 products on 22 chains) vs DVE.
CONFIG_A = _spread({"AAVV-": 22, "VVVV-": 2})
# Config B (with GPSIMD adds/finals).
CONFIG_B = _spread({"AVVV-": 8, "AAVV-": 6, "AAPV-": 6, "AVVPV": 4})

CHAIN_CODES = CONFIG_A


def build_bass(reps: int = 1, ops_mode: str = "full", chain_codes=None):
    if chain_codes is None:
        chain_codes = CHAIN_CODES
    nc = bacc.Bacc(
        "TRN2",
        debug=False,
        enable_asserts=False,
        target_bir_lowering=False,
        num_devices=N_CORES,
    )
    f32 = mybir.dt.float32
    bf16 = mybir.dt.bfloat16
    mult = mybir.AluOpType.mult
    add = mybir.AluOpType.add
    copy = mybir.ActivationFunctionType.Copy

    # xs[p = a*SEGS + s, col = g*9 + i*3 + j] = x[b, g*8 + a, i, j]
    xs = nc.dram_tensor("xs", [P, N_GROUPS * 9], f32, kind="ExternalInput").ap()
    # yt[j, m] = y[b, m, j]  (bf16 planes)
    yt = nc.dram_tensor("yt", [3, M], bf16, kind="ExternalInput").ap()
    out = nc.dram_tensor("out", [N_PER_CORE, M, 3], f32, kind="ExternalOutput").ap()

    with TileContext(nc) as tc:
        with (
            tc.tile_pool(name="const", bufs=1) as cpool,
            tc.tile_pool(name="tmpp", bufs=2) as tpool,
            tc.tile_pool(name="outp", bufs=2) as opool,
        ):
            xsb = cpool.tile([P, N_GROUPS * 9], f32)
            nc.sync.dma_start(out=xsb[:], in_=xs)

            # y resident in SBUF as bf16 planes: partition (a, s) holds
            # [j, t] = yt[j, s*ROWS + t]; replicated over the 8 a-groups.
            # Loaded in halves so unit (g=0, h=0) can start early.
            y_tile = cpool.tile([P, 3 * ROWS], bf16)
            yv = y_tile.rearrange("p (j t) -> p j t", j=3)
            y_src = yt.rearrange("j (s t) -> j s t", s=SEGS)  # [3, 16, 6250]
            for h in range(N_HALVES):
                t0, t1 = BOUNDS[h], BOUNDS[h + 1]
                for j in range(3):
                    nc.sync.dma_start(
                        out=yv[:, j, t0:t1],
                        in_=y_src[j, :, t0:t1].unsqueeze(0)
                        .to_broadcast((N_SUB, SEGS, t1 - t0)),
                    )

            units = [(g, h) for g in range(N_GROUPS) for h in range(N_HALVES)]
            units = units * reps

            def emit_unit(u, g, h):
                t0, t1 = BOUNDS[h], BOUNDS[h + 1]
                nt = t1 - t0
                ot = opool.tile([P, nt * 3], f32, name="ot", tag="ot")
                ov = ot.rearrange("p (t i) -> p t i", i=3)
                ys = [yv[:, j, t0:t1] for j in range(3)]
                prods = []
                for i in range(3):
                    code = chain_codes[((g * N_HALVES + h) * 3 + i) % 24]
                    c = g * 9 + i * 3
                    a = tpool.tile([P, nt], bf16, name=f"a{i}", tag=f"a{i}")
                    b = tpool.tile([P, nt], bf16, name=f"b{i}", tag=f"b{i}")
                    for slot, (eng, j, dst) in enumerate(
                        [(code[0], 0, a), (code[1], 1, b)]
                    ):
                        if eng == "A":
                            nc.scalar.activation(
                                out=dst[:], in_=ys[j], func=copy,
                                scale=xsb[:, c + j:c + j + 1])
                        else:
                            nc.vector.tensor_scalar(
                                out=dst[:], in0=ys[j],
                                scalar1=xsb[:, c + j:c + j + 1], scalar2=None,
                                op0=mult)
                    prods.append((code, c, a, b))
                for i in range(3):
                    code, c, a, b = prods[i]
                    if code[2] == "V":
                        nc.vector.tensor_tensor(out=a[:], in0=a[:], in1=b[:],
                                                op=add)
                    else:
                        nc.gpsimd.tensor_tensor(out=a[:], in0=a[:], in1=b[:],
                                                op=add)
                for i in range(3):
                    code, c, a, b = prods[i]
                    if code[3] == "V":
                        # out_i = (y2 * x2) + (j0 + j1): fuses the third
                        # product into the strided fp32 interleave write.
                        nc.vector.scalar_tensor_tensor(
                            out=ov[:, :, i], in0=ys[2],
                            scalar=xsb[:, c + 2:c + 3], in1=a[:],
                            op0=mult, op1=add)
                        continue
                    b2 = tpool.tile([P, nt], bf16, name=f"c{i}", tag=f"c{i}",
                                    bufs=1)
                    if code[4] == "A":
                        nc.scalar.activation(
                            out=b2[:], in_=ys[2], func=copy,
                            scale=xsb[:, c + 2:c + 3])
                    else:
                        nc.vector.tensor_scalar(
                            out=b2[:], in0=ys[2],
                            scalar1=xsb[:, c + 2:c + 3], scalar2=None,
                            op0=mult)
                    if code[3] == "P":
                        # GPSIMD does add + interleave in one op.
                        nc.gpsimd.tensor_tensor(
                            out=ov[:, :, i], in0=a[:], in1=b2[:], op=add)
                    else:
                        # bf16 2x add, then 1x strided upcast copy off-DVE.
                        nc.vector.tensor_tensor(out=a[:], in0=a[:], in1=b2[:],
                                                op=add)
                        if code[3] == "A":
                            nc.scalar.activation(out=ov[:, :, i], in_=a[:],
                                                 func=copy)
                        else:
                            nc.gpsimd.tensor_copy(out=ov[:, :, i], in_=a[:])
                dst = out[g * N_SUB:(g + 1) * N_SUB, :, :].rearrange(
                    "a (s t) i -> (a s) t i", s=SEGS
                )[:, t0:t1, :]
                nc.sync.dma_start(out=dst, in_=ot[:])

            def emit_unit_none(u, g, h):
                t0, t1 = BOUNDS[h], BOUNDS[h + 1]
                nt = t1 - t0
                ot = opool.tile([P, nt * 3], f32, name="ot", tag="ot")
                nc.vector.memset(ot[:], 0.0)
                dst = out[g * N_SUB:(g + 1) * N_SUB, :, :].rearrange(
                    "a (s t) i -> (a s) t i", s=SEGS
                )[:, t0:t1, :]
                nc.sync.dma_start(out=dst, in_=ot[:])

            for u, (g, h) in enumerate(units):
                if ops_mode == "full":
                    emit_unit(u, g, h)
                elif ops_mode == "none":
                    emit_unit_none(u, g, h)
                else:
                    raise ValueError(ops_mode)
    nc.compile()
    return nc


def _make_in_maps(x, y):
    x_flat = x.reshape(256, 3, 3)
    in_maps = []
    for c in range(N_CORES):
        b = c // 2
        xl = x_flat[c * N_PER_CORE:(c + 1) * N_PER_CORE]  # [32, 3, 3]
        per_a = xl.reshape(N_GROUPS, N_SUB, 9).transpose(1, 0, 2)  # [a, g, 9]
        per_a = per_a.reshape(N_SUB, N_GROUPS * 9)
        xsb_np = np.ascontiguousarray(np.repeat(per_a, SEGS, axis=0))  # [128, 36]
        yt_np = np.ascontiguousarray(y[b].T).astype(ml_dtypes.bfloat16)
        in_maps.append({"xs": xsb_np, "yt": yt_np})
    return in_maps


def kernel(x: np.ndarray, y: np.ndarray) -> np.ndarray:
    global LAST, _CACHED_NC
    x = np.ascontiguousarray(x, dtype=np.float32)
    y = np.ascontiguousarray(y, dtype=np.float32)
    assert x.shape == (4, 64, 3, 3) and y.shape == (4, 100000, 3)

    if _CACHED_NC is None:
        _CACHED_NC = build_bass()
    nc = _CACHED_NC

    in_maps = _make_in_maps(x, y)
    res = run_bass_kernel_spmd(
        nc, in_maps, core_ids=list(range(N_CORES)), trace=TRACE,
    )
    LAST = res
    out = np.concatenate([r["out"] for r in res.results], axis=0)
    return out.reshape(4, 64, 100000, 3)


def _prepare_exec(nc, in_maps):
    """Build a jitted 8-core executor for `nc` with device-resident inputs."""
    import jax
    import concourse.mybir as mybir_
    from jax.experimental.shard_map import shard_map
    from jax.sharding import Mesh, NamedSharding, PartitionSpec
    from concourse.bass2jax import (
        _bass_exec_p, install_neuronx_cc_hook, partition_id_tensor,
    )

    install_neuronx_cc_hook()
    partition_name = nc.partition_id_tensor.name if nc.partition_id_tensor else None
    in_names, out_names, out_avals, zero_outs = [], [], [], []
    for alloc in nc.m.functions[0].allocations:
        if not isinstance(alloc, mybir_.MemoryLocationSet):
            continue
        name = alloc.memorylocations[0].name
        if alloc.kind == "ExternalInput":
            if name != partition_name:
                in_names.append(name)
        elif alloc.kind == "ExternalOutput":
            shape = tuple(alloc.tensor_shape)
            dtype = mybir_.dt.np(alloc.dtype)
            out_names.append(name)
            out_avals.append(jax.core.ShapedArray(shape, dtype))
            zero_outs.append(np.zeros(shape, dtype))
    n_params = len(in_names)
    n_outs = len(out_names)
    all_names = in_names + out_names + ([partition_name] if partition_name else [])

    def _body(*args):
        operands = list(args)
        if partition_name is not None:
            operands.append(partition_id_tensor())
        outs = _bass_exec_p.bind(
            *operands,
            out_avals=tuple(out_avals),
            in_names=tuple(all_names),
            out_names=tuple(out_names),
            lowering_input_output_aliases=(),
            sim_require_finite=True,
            sim_require_nnan=True,
            nc=nc,
        )
        return tuple(outs)

    devices = jax.devices()[:N_CORES]
    mesh = Mesh(np.asarray(devices), ("core",))
    spec = PartitionSpec("core")
    sharded = jax.jit(
        shard_map(
            _body, mesh=mesh, in_specs=(spec,) * (n_params + n_outs),
            out_specs=(spec,) * n_outs, check_rep=False,
        ),
        donate_argnums=tuple(range(n_params, n_params + n_outs)),
        keep_unused=True,
    )
    sh = NamedSharding(mesh, spec)
    ins_dev = [
        jax.device_put(
            np.concatenate([np.asarray(m[name]) for m in in_maps], axis=0), sh
        )
        for name in in_names
    ]
    zeros = [
        jax.device_put(
            np.zeros((N_CORES * z.shape[0], *z.shape[1:]), z.dtype), sh
        )
        for z in zero_outs
    ]

    def run_once(outs):
        res = sharded(*ins_dev, *outs)
        jax.block_until_ready(res)
        return list(res)

    return run_once, zeros


def bench(x, y, reps_pair=(9, 65), samples=24, ops_mode="full", chain_codes=None):
    """Measure steady-state per-workload HW time by differencing kernels
    that run the workload `reps_pair[0]` vs `reps_pair[1]` times.

    Wall-clock per call is dominated by host/tunnel dispatch noise (tens of
    ms, multi-modal), dwarfing the ~1-8 ms device time. To recover the
    device-side scaling with reps we time BATCHES of consecutive
    executions (amortizing per-call jitter) and difference the per-call
    batch minima between the two reps counts."""
    import time
    x = np.ascontiguousarray(x, dtype=np.float32)
    y = np.ascontiguousarray(y, dtype=np.float32)
    in_maps = _make_in_maps(x, y)
    batch_k = 5
    n_batches = max(6, samples // 2)
    mins = {}
    for reps in reps_pair:
        nc = build_bass(reps=reps, ops_mode=ops_mode, chain_codes=chain_codes)
        run_once, zeros = _prepare_exec(nc, in_maps)
        outs = run_once(zeros)  # compile + warm
        outs = run_once(outs)
        bs = []
        for _ in range(n_batches):
            t0 = time.perf_counter()
            for _ in range(batch_k):
                outs = run_once(outs)
            bs.append((time.perf_counter() - t0) / batch_k)
        bs.sort()
        mins[reps] = bs[0]
        print(f"reps={reps}: per-call batch-min {bs[0]*1e3:.2f} ms  "
              f"batches {[f'{t*1e3:.1f}' for t in bs]}")
    r1, r2 = reps_pair
    per_iter = (mins[r2] - mins[r1]) / (r2 - r1) * 1e9
    print(f"per-iter (batch-min diff): {per_iter:.0f} ns")
    return per_iter
